# revision 5
# baseline (speedup 1.0000x reference)
"""DelayLMLIFLayer Trainium2 kernel.

Pipeline per core (8 cores, 4-way I-shard x 2-way B-shard):
  1. DCLS delayed conv main term: 16 time-shifted fp32r matmuls per chunk,
     PSUM-accumulated; doubles as the BatchNorm stats source (accum_out).
  2. BN stats: pairwise AllGather over b-half pairs + local add.
  3. Cross-term correction in ONE fp8e4 DoubleRow pass: each tap matmul
     computes wl@xh + wh@xl simultaneously (pair-packed operands, x2^11
     scaling to keep the low parts in fp8 range), at 0.5 cycles/row.
  4. Scan runs in v = U/beta space, 2 DVE instrs per step:
       d_t = (v >= 1/beta) - A2_t        (= S_{t-1} - A2_t)
       v'  = beta*v - d_t                (= beta*v - S_{t-1} + A2_t)
     Spikes are recovered off the critical path on Pool:
       S_{t-1} = ((d_t + A2_t) >= 0.5), exact {0,1}.
Host does layout transposes, fp32r/fp8 splits, and beta-space folds.
"""
import sys
sys.path.insert(0, '/opt/trn_rl_repo')

import numpy as np

T, B, J, I, KD = 1024, 32, 128, 512, 16
SIG = 0.5
EPS = 1e-5
N_CORES = 8
BH = B // 2          # batch elems per core (b-half)
IC = 128             # channels per core (I-chunk)
ROWS = T * BH        # free-dim rows per core
PAD = (KD - 1) * BH  # left zero pad columns (240)
CHUNK = 512          # psum tile free size
NCH = ROWS // CHUNK  # 32 row chunks
TPC = CHUNK // BH    # 32 timesteps per chunk
SC = 2.0 ** 11       # fp8 low-part scale

_CACHE = {}


def _to_fp32r(x):
    u = np.ascontiguousarray(x, np.float32).view(np.uint32).astype(np.uint64)
    rnd = ((u >> 12) & 1) + 0x7FF
    u = ((u + rnd) >> 12) << 12
    return (u & 0xFFFFFFFF).astype(np.uint32).view(np.float32)


def _build_nc():
    import concourse.bacc as bacc
    import concourse.mybir as mybir
    import concourse.tile as tile

    F32 = mybir.dt.float32
    F32R = mybir.dt.float32r
    F8 = mybir.dt.float8e4
    OP = mybir.AluOpType
    ACT = mybir.ActivationFunctionType

    nc = bacc.Bacc("TRN2", target_bir_lowering=False, debug=False,
                   num_devices=N_CORES)

    xh_d = nc.dram_tensor("xh", [J, ROWS], F32, kind="ExternalInput")
    xp_d = nc.dram_tensor("xp", [J, 2, ROWS], F8, kind="ExternalInput")
    whr_d = nc.dram_tensor("whr", [J, KD * IC], F32, kind="ExternalInput")
    w8_d = nc.dram_tensor("w8", [J, KD, 2, IC], F8, kind="ExternalInput")
    v0_d = nc.dram_tensor("v0", [IC, BH], F32, kind="ExternalInput")
    pch_d = nc.dram_tensor("pch", [IC, 4], F32, kind="ExternalInput")
    sout_d = nc.dram_tensor("sout", [IC, ROWS], F32, kind="ExternalOutput")

    with tile.TileContext(nc) as tc:
        with (
            tc.tile_pool(name="big", bufs=1) as big,
            tc.tile_pool(name="xs", bufs=3) as xs,
            tc.tile_pool(name="ost", bufs=2) as ostp,
            tc.tile_pool(name="small", bufs=1) as small,
            tc.tile_pool(name="ps", bufs=4, space="PSUM") as ps,
            tc.tile_pool(name="dram", bufs=1, space="DRAM") as dram,
        ):
            At = [big.tile([IC, CHUNK], F32, tag=f"A{r}", name=f"A{r}")
                  for r in range(NCH)]
            St = [big.tile([IC, CHUNK], F32, tag=f"S{r}", name=f"S{r}")
                  for r in range(NCH)]
            scr = big.tile([IC, CHUNK], F32, tag="scr")
            whr = small.tile([J, KD * IC], F32R, tag="whr")
            w8 = small.tile([J, KD, 2, IC], F8, tag="w8")
            pch = small.tile([IC, 4], F32, tag="pch")
            Vc = small.tile([IC, BH], F32, tag="Vc")
            ssum = small.tile([IC, NCH], F32, tag="ssum")
            ssq = small.tile([IC, NCH], F32, tag="ssq")
            st2 = small.tile([IC, 2], F32, tag="st2")
            gs4 = small.tile([IC, 4], F32, tag="gs4")
            gs = small.tile([IC, 2], F32, tag="gs")
            prm = small.tile([IC, 8], F32, tag="prm")

            cc_in = dram.tile([IC, 2], F32)
            cc_out = dram.tile([2, IC, 2], F32)

            beta = pch[:, 0:1]
            thr = pch[:, 1:2]
            g2 = pch[:, 2:3]
            bb2 = pch[:, 3:4]

            # pass-1 weights in two halves so taps 0-7 gate sooner
            nc.sync.dma_start(whr[:, :8 * IC], whr_d[:, :8 * IC].bitcast(F32R))
            nc.sync.dma_start(whr[:, 8 * IC:], whr_d[:, 8 * IC:].bitcast(F32R))
            nc.sync.dma_start(Vc[:], v0_d[:])
            nc.sync.dma_start(pch[:], pch_d[:])
            nc.sync.dma_start(w8[:], w8_d[:])

            # ---- conv pass 1: fp32r main term; doubles as the BN stats source ----
            for r in range(NCH):
                c0 = r * CHUNK - PAD
                xh_c = xs.tile([J, PAD + CHUNK], F32R, tag="xh_c")
                if r == 0:
                    nc.vector.memset(xh_c[:, :PAD].bitcast(F32), 0.0)
                    nc.sync.dma_start(xh_c[:, PAD:], xh_d[:, 0:CHUNK].bitcast(F32R))
                else:
                    nc.sync.dma_start(xh_c[:], xh_d[:, c0:c0 + PAD + CHUNK].bitcast(F32R))

                pt = ps.tile([IC, CHUNK], F32, tag="pt")
                for k in range(KD):
                    nc.tensor.matmul(pt[:], whr[:, k * IC:(k + 1) * IC],
                                     xh_c[:, k * BH:k * BH + CHUNK],
                                     start=(k == 0), stop=(k == KD - 1))

                nc.scalar.activation(At[r][:], pt[:], ACT.Copy,
                                     accum_out=ssum[:, r:r + 1])
                nc.scalar.activation(scr[:], pt[:], ACT.Square,
                                     accum_out=ssq[:, r:r + 1])

            # ---- BN stats allreduce over the b-half pair ----
            nc.vector.tensor_reduce(st2[:, 0:1], ssum[:], mybir.AxisListType.X, OP.add)
            nc.vector.tensor_reduce(st2[:, 1:2], ssq[:], mybir.AxisListType.X, OP.add)
            nc.sync.dma_start(cc_in[:], st2[:])
            # AllGather + local add == AllReduce (add is commutative) at
            # roughly half the fixed latency.
            nc.gpsimd.collective_compute(
                "AllGather", OP.bypass,
                replica_groups=[[0, 1], [2, 3], [4, 5], [6, 7]],
                ins=[cc_in.opt()], outs=[cc_out.opt()],
            )
            nc.sync.dma_start(gs4[:, 0:2], cc_out[0, :, :])
            nc.sync.dma_start(gs4[:, 2:4], cc_out[1, :, :])
            nc.vector.tensor_tensor(gs[:], gs4[:, 0:2], gs4[:, 2:4], OP.add)

            # ---- fold BN + (1-beta) + 1/beta into per-channel av2, bv2 ----
            inv_n = 1.0 / (T * B)
            mean = prm[:, 0:1]; ey2 = prm[:, 1:2]; var = prm[:, 2:3]
            rs = prm[:, 3:4]; av2 = prm[:, 4:5]; bv2 = prm[:, 5:6]
            tmp = prm[:, 6:7]
            nc.vector.tensor_scalar(mean, gs[:, 0:1], inv_n, None, OP.mult)
            nc.vector.tensor_scalar(ey2, gs[:, 1:2], inv_n, None, OP.mult)
            nc.vector.tensor_tensor(tmp, mean, mean, OP.mult)
            nc.vector.tensor_tensor(var, ey2, tmp, OP.subtract)
            nc.vector.tensor_scalar(var, var, EPS, None, OP.add)
            nc.scalar.sqrt(tmp, var)
            nc.vector.reciprocal(rs, tmp)
            nc.vector.tensor_tensor(av2, g2, rs, OP.mult)       # av2 = (1-b)g/(b*sigma)
            nc.vector.tensor_tensor(tmp, av2, mean, OP.mult)
            nc.vector.tensor_tensor(bv2, bb2, tmp, OP.subtract)  # bv2 = bb2 - av2*mean

            # ---- conv pass 2: fp8 DoubleRow cross terms + combine + affine.
            # Runs on PE/ACT/Pool concurrently with the DVE scan below.
            for r in range(NCH):
                c0 = r * CHUNK - PAD
                xp_c = xs.tile([J, 2, PAD + CHUNK], F8, tag="xp_c")
                if r == 0:
                    nc.vector.memset(xp_c[:, :, :PAD], 0.0)
                    nc.sync.dma_start(xp_c[:, :, PAD:], xp_d[:, :, 0:CHUNK])
                else:
                    nc.sync.dma_start(xp_c[:], xp_d[:, :, c0:c0 + PAD + CHUNK])

                pt2 = ps.tile([IC, CHUNK], F32, tag="pt2")
                for k in range(KD):
                    nc.tensor.matmul(pt2[:], w8[:, k, :, :],
                                     xp_c[:, :, k * BH:k * BH + CHUNK],
                                     start=(k == 0), stop=(k == KD - 1),
                                     perf_mode=mybir.MatmulPerfMode.DoubleRow)

                s2 = xs.tile([IC, CHUNK], F32, tag="s2")
                sl = At[r][:]
                nc.scalar.activation(s2[:], pt2[:], ACT.Copy, scale=float(1.0 / SC))
                nc.gpsimd.tensor_tensor(sl, sl, s2[:], OP.add)
                nc.gpsimd.tensor_scalar(sl, sl, av2, bv2, OP.mult, OP.add)

            # ---- LIF scan in v = U/beta space: 2 DVE instrs per step ----
            for t in range(T):
                rt, lt = t // TPC, (t % TPC) * BH
                d = St[rt][:, lt:lt + BH]
                a2 = At[rt][:, lt:lt + BH]
                nc.vector.scalar_tensor_tensor(d, Vc[:], thr, a2,
                                               OP.is_ge, OP.subtract)
                nc.vector.scalar_tensor_tensor(Vc[:], Vc[:], beta, d,
                                               OP.mult, OP.subtract)

            # ---- spike recovery + output: S_{t-1} = (d_t + A2_t >= 0.5) ----
            # Runs on Pool, trailing the scan by one chunk; exact {0,1} out.
            for r in range(NCH):
                O = ostp.tile([IC, CHUNK], F32, tag="ost")
                nc.gpsimd.tensor_tensor(O[:, 0:CHUNK - BH], St[r][:, BH:],
                                        At[r][:, BH:], OP.add)
                if r < NCH - 1:
                    nc.gpsimd.tensor_tensor(O[:, CHUNK - BH:], St[r + 1][:, 0:BH],
                                            At[r + 1][:, 0:BH], OP.add)
                    nc.gpsimd.tensor_scalar(O[:], O[:], 0.5, None, OP.is_ge)
                else:
                    nc.gpsimd.tensor_scalar(O[:, 0:CHUNK - BH], O[:, 0:CHUNK - BH],
                                            0.5, None, OP.is_ge)
                    # closing spike s_{T-1} = (v_{T-1} >= thr), exact
                    nc.vector.tensor_scalar(O[:, CHUNK - BH:], Vc[:], thr, None,
                                            OP.is_ge)
                nc.sync.dma_start(sout_d[:, r * CHUNK:(r + 1) * CHUNK], O[:])

    nc.finalize()
    return nc


def _prep_inputs(x, delay_w, delay_P, beta, bn_gamma, bn_beta, U0):
    import ml_dtypes
    f8 = ml_dtypes.float8_e4m3
    c = (delay_P.astype(np.float32) + KD // 2)
    k = np.arange(KD, dtype=np.float32)
    g = np.exp(-0.5 * ((k[None, None, :] - c[:, :, None]) / SIG) ** 2).astype(np.float32)
    g = g / (g.sum(-1, keepdims=True) + np.float32(1e-7))
    kern = (delay_w.astype(np.float32)[:, :, None] * g).astype(np.float32)  # (I,J,KD)

    kh = _to_fp32r(kern)
    kl = (kern - kh).astype(np.float32)
    xh = _to_fp32r(x)
    xl = (x - xh).astype(np.float32)

    kh_jki = np.ascontiguousarray(kh.transpose(1, 2, 0))       # (J,KD,I) f32
    kl8s = np.ascontiguousarray((kl * SC).transpose(1, 2, 0)).astype(f8)
    kh8 = kh_jki.astype(f8)

    xt_h = np.ascontiguousarray(xh.transpose(2, 0, 1))         # (J,T,B) f32
    xh8 = xt_h.astype(f8)
    xl8s = np.ascontiguousarray((xl * SC).transpose(2, 0, 1)).astype(f8)

    rb = (1.0 / beta).astype(np.float32)
    g2_full = ((1.0 - beta) * bn_gamma * rb).astype(np.float32)
    bb2_full = ((1.0 - beta) * bn_beta * rb).astype(np.float32)

    in_maps = []
    for core in range(N_CORES):
        gi, hi = core // 2, core % 2
        isl = slice(gi * IC, (gi + 1) * IC)
        bsl = slice(hi * BH, (hi + 1) * BH)
        pch = np.stack([beta[isl], rb[isl], g2_full[isl], bb2_full[isl]], axis=1)
        w8 = np.empty((J, KD, 2, IC), f8)
        w8[:, :, 0, :] = kl8s[:, :, isl]
        w8[:, :, 1, :] = kh8[:, :, isl]
        xp = np.empty((J, 2, ROWS), f8)
        xp[:, 0, :] = xh8[:, :, bsl].reshape(J, ROWS)
        xp[:, 1, :] = xl8s[:, :, bsl].reshape(J, ROWS)
        in_maps.append({
            "xh": np.ascontiguousarray(xt_h[:, :, bsl]).reshape(J, ROWS),
            "xp": xp,
            "whr": np.ascontiguousarray(kh_jki[:, :, isl]).reshape(J, KD * IC),
            "w8": np.ascontiguousarray(w8),
            "v0": np.ascontiguousarray((U0[bsl, isl] * rb[None, isl]).T.astype(np.float32)),
            "pch": np.ascontiguousarray(pch.astype(np.float32)),
        })
    return in_maps


def run_spmd(in_maps, **kwargs):
    from concourse.bass_utils import run_bass_kernel_spmd
    if "nc" not in _CACHE:
        _CACHE["nc"] = _build_nc()
    return run_bass_kernel_spmd(_CACHE["nc"], in_maps,
                                core_ids=list(range(N_CORES)), **kwargs)


def kernel(x, delay_w, delay_P, beta, bn_gamma, bn_beta, U0):
    in_maps = _prep_inputs(np.asarray(x, np.float32), np.asarray(delay_w, np.float32),
                           np.asarray(delay_P, np.float32), np.asarray(beta, np.float32),
                           np.asarray(bn_gamma, np.float32), np.asarray(bn_beta, np.float32),
                           np.asarray(U0, np.float32))
    res = run_spmd(in_maps)
    out = np.empty((T, B, I), np.float32)
    for core in range(N_CORES):
        gi, hi = core // 2, core % 2
        s = res.results[core]["sout"].reshape(IC, T, BH)
        out[:, hi * BH:(hi + 1) * BH, gi * IC:(gi + 1) * IC] = s.transpose(1, 2, 0)
    return out


# revision 11
# speedup vs baseline: 1.0739x; 1.0739x over previous
"""DelayLMLIFLayer Trainium2 kernel.

Pipeline per core (8 cores, 4-way I-shard x 2-way B-shard):
  1. DCLS delayed conv main term: 16 time-shifted fp32r matmuls per chunk,
     PSUM-accumulated; doubles as the BatchNorm stats source (accum_out).
  2. BN stats: pairwise AllGather over b-half pairs + local add.
  3. Cross-term correction in ONE fp8e4 DoubleRow pass: each tap matmul
     computes wl@xh + wh@xl simultaneously (pair-packed operands, x2^11
     scaling to keep the low parts in fp8 range), at 0.5 cycles/row.
  4. Scan runs in v = U/beta space, 2 DVE instrs per step per b-half chain:
       d_t = (v >= 1/beta) - A2_t        (= S_{t-1} - A2_t)
       v'  = beta*v - d_t                (= beta*v - S_{t-1} + A2_t)
     Two 8-wide chains interleave so every producer is 2 instructions back,
     hiding the SBUF write-ack + semaphore latency of each RAW edge.
     Spikes are recovered off the critical path on Pool:
       S_{t-1} = ((d_t + A2_t) >= 0.5), exact {0,1}.
  5. A burst of dummy matmuls during the initial weight-DMA shadow absorbs
     the PE p-state ramp so pass-1 runs at full clock throughout.
Host does layout transposes, fp32r/fp8 splits, and beta-space folds.
"""
import sys
sys.path.insert(0, '/opt/trn_rl_repo')

import numpy as np

T, B, J, I, KD = 1024, 32, 128, 512, 16
SIG = 0.5
EPS = 1e-5
N_CORES = 8
BH = B // 2          # batch elems per core (b-half)
IC = 128             # channels per core (I-chunk)
ROWS = T * BH        # free-dim rows per core
PAD = (KD - 1) * BH  # left zero pad columns (240)
CHUNK = 512          # psum tile free size
NCH = ROWS // CHUNK  # 32 row chunks
TPC = CHUNK // BH    # 32 timesteps per chunk
SC = 2.0 ** 11       # fp8 low-part scale

_CACHE = {}


def _to_fp32r(x):
    u = np.ascontiguousarray(x, np.float32).view(np.uint32).astype(np.uint64)
    rnd = ((u >> 12) & 1) + 0x7FF
    u = ((u + rnd) >> 12) << 12
    return (u & 0xFFFFFFFF).astype(np.uint32).view(np.float32)


def _build_nc():
    import concourse.bacc as bacc
    import concourse.mybir as mybir
    import concourse.tile as tile

    F32 = mybir.dt.float32
    F32R = mybir.dt.float32r
    F8 = mybir.dt.float8e4
    BF16 = mybir.dt.bfloat16
    OP = mybir.AluOpType
    ACT = mybir.ActivationFunctionType

    nc = bacc.Bacc("TRN2", target_bir_lowering=False, debug=False,
                   num_devices=N_CORES)

    xh_d = nc.dram_tensor("xh", [J, ROWS], F32, kind="ExternalInput")
    xp_d = nc.dram_tensor("xp", [J, 2, ROWS], F8, kind="ExternalInput")
    whr_d = nc.dram_tensor("whr", [J, KD * IC], F32, kind="ExternalInput")
    w8_d = nc.dram_tensor("w8", [J, KD, 2, IC], F8, kind="ExternalInput")
    v0_d = nc.dram_tensor("v0", [IC, BH], F32, kind="ExternalInput")
    pch_d = nc.dram_tensor("pch", [IC, 4], F32, kind="ExternalInput")
    sout_d = nc.dram_tensor("sout", [IC, ROWS], F32, kind="ExternalOutput")

    with tile.TileContext(nc) as tc:
        with (
            tc.tile_pool(name="big", bufs=1) as big,
            tc.tile_pool(name="xs", bufs=3) as xs,
            tc.tile_pool(name="ost", bufs=2) as ostp,
            tc.tile_pool(name="small", bufs=1) as small,
            tc.tile_pool(name="ps", bufs=4, space="PSUM") as ps,
            tc.tile_pool(name="dram", bufs=1, space="DRAM") as dram,
        ):
            At = [big.tile([IC, CHUNK], F32, tag=f"A{r}", name=f"A{r}")
                  for r in range(NCH)]
            St = [big.tile([IC, CHUNK], F32, tag=f"S{r}", name=f"S{r}")
                  for r in range(NCH)]
            scr = big.tile([IC, CHUNK], F32, tag="scr")
            whr = small.tile([J, KD * IC], F32R, tag="whr")
            w8 = small.tile([J, KD, 2, IC], F8, tag="w8")
            pch = small.tile([IC, 4], F32, tag="pch")
            Vc = small.tile([IC, BH], F32, tag="Vc")
            ssum = small.tile([IC, NCH], F32, tag="ssum")
            ssq = small.tile([IC, NCH], F32, tag="ssq")
            st2 = small.tile([IC, 2], F32, tag="st2")
            gs4 = small.tile([IC, 4], F32, tag="gs4")
            gs = small.tile([IC, 2], F32, tag="gs")
            prm = small.tile([IC, 8], F32, tag="prm")

            cc_in = dram.tile([IC, 2], F32)
            cc_out = dram.tile([2, IC, 2], F32)

            beta = pch[:, 0:1]
            thr = pch[:, 1:2]
            g2 = pch[:, 2:3]
            bb2 = pch[:, 3:4]

            # pass-1 weights in two halves so taps 0-7 gate sooner
            nc.sync.dma_start(whr[:, :8 * IC], whr_d[:, :8 * IC].bitcast(F32R))
            nc.sync.dma_start(whr[:, 8 * IC:], whr_d[:, 8 * IC:].bitcast(F32R))

            # PE p-state warmup: dummy matmuls on zeroed tiles burn the
            # low/mid-clock ramp inside the initial DMA shadow. The first 8
            # run 512 cols (>3us of engine time); the rest are 2-col fillers
            # that keep the PE queue full so every real matmul is costed at
            # the ramped clock.
            wz = small.tile([J, 2], BF16, tag="wz")
            xz = small.tile([J, CHUNK], BF16, tag="xz")
            nc.vector.memset(wz[:], 0.0)
            nc.vector.memset(xz[:], 0.0)
            ptw = ps.tile([IC, CHUNK], F32, tag="pt")
            for i in range(8):
                nc.tensor.matmul(ptw[0:2, :], wz[:], xz[:], start=True, stop=True)
            for i in range(30):
                nc.tensor.matmul(ptw[0:2, 0:2], wz[:], xz[:, 0:2],
                                 start=True, stop=True)

            # ---- conv pass 1: fp32r main term; doubles as the BN stats source ----
            for r in range(NCH):
                c0 = r * CHUNK - PAD
                xh_c = xs.tile([J, PAD + CHUNK], F32R, tag="xh_c")
                if r == 0:
                    nc.vector.memset(xh_c[:, :PAD].bitcast(F32), 0.0)
                    nc.sync.dma_start(xh_c[:, PAD:], xh_d[:, 0:CHUNK].bitcast(F32R))
                else:
                    nc.sync.dma_start(xh_c[:], xh_d[:, c0:c0 + PAD + CHUNK].bitcast(F32R))
                if r == 1:
                    # scan/pass-2 constants ride behind the first two x chunks
                    nc.sync.dma_start(Vc[:], v0_d[:])
                    nc.sync.dma_start(pch[:], pch_d[:])
                    nc.sync.dma_start(w8[:], w8_d[:])

                pt = ps.tile([IC, CHUNK], F32, tag="pt")
                for k in range(KD):
                    nc.tensor.matmul(pt[:], whr[:, k * IC:(k + 1) * IC],
                                     xh_c[:, k * BH:k * BH + CHUNK],
                                     start=(k == 0), stop=(k == KD - 1))

                nc.scalar.activation(At[r][:], pt[:], ACT.Copy,
                                     accum_out=ssum[:, r:r + 1])
                nc.scalar.activation(scr[:], pt[:], ACT.Square,
                                     accum_out=ssq[:, r:r + 1])

            # ---- BN stats allreduce over the b-half pair ----
            nc.vector.tensor_reduce(st2[:, 0:1], ssum[:], mybir.AxisListType.X, OP.add)
            nc.vector.tensor_reduce(st2[:, 1:2], ssq[:], mybir.AxisListType.X, OP.add)
            nc.sync.dma_start(cc_in[:], st2[:])
            # AllGather + local add == AllReduce (add is commutative) at
            # roughly half the fixed latency.
            nc.gpsimd.collective_compute(
                "AllGather", OP.bypass,
                replica_groups=[[0, 1], [2, 3], [4, 5], [6, 7]],
                ins=[cc_in.opt()], outs=[cc_out.opt()],
            )
            nc.sync.dma_start(gs4[:, 0:2], cc_out[0, :, :])
            nc.sync.dma_start(gs4[:, 2:4], cc_out[1, :, :])
            nc.vector.tensor_tensor(gs[:], gs4[:, 0:2], gs4[:, 2:4], OP.add)

            # ---- fold BN + (1-beta) + 1/beta into per-channel av2, bv2 ----
            inv_n = 1.0 / (T * B)
            mean = prm[:, 0:1]; ey2 = prm[:, 1:2]; var = prm[:, 2:3]
            rs = prm[:, 3:4]; av2 = prm[:, 4:5]; bv2 = prm[:, 5:6]
            tmp = prm[:, 6:7]
            nc.vector.tensor_scalar(mean, gs[:, 0:1], inv_n, None, OP.mult)
            nc.vector.tensor_scalar(ey2, gs[:, 1:2], inv_n, None, OP.mult)
            nc.vector.tensor_tensor(tmp, mean, mean, OP.mult)
            nc.vector.tensor_tensor(var, ey2, tmp, OP.subtract)
            nc.vector.tensor_scalar(var, var, EPS, None, OP.add)
            nc.scalar.sqrt(tmp, var)
            nc.vector.reciprocal(rs, tmp)
            nc.vector.tensor_tensor(av2, g2, rs, OP.mult)       # av2 = (1-b)g/(b*sigma)
            nc.vector.tensor_tensor(tmp, av2, mean, OP.mult)
            nc.vector.tensor_tensor(bv2, bb2, tmp, OP.subtract)  # bv2 = bb2 - av2*mean

            # ---- conv pass 2: fp8 DoubleRow cross terms + combine + affine.
            # Runs on PE/ACT/Pool concurrently with the DVE scan below.
            for r in range(NCH):
                c0 = r * CHUNK - PAD
                xp_c = xs.tile([J, 2, PAD + CHUNK], F8, tag="xp_c")
                if r == 0:
                    nc.vector.memset(xp_c[:, :, :PAD], 0.0)
                    nc.sync.dma_start(xp_c[:, :, PAD:], xp_d[:, :, 0:CHUNK])
                else:
                    nc.sync.dma_start(xp_c[:], xp_d[:, :, c0:c0 + PAD + CHUNK])

                pt2 = ps.tile([IC, CHUNK], F32, tag="pt2")
                for k in range(KD):
                    nc.tensor.matmul(pt2[:], w8[:, k, :, :],
                                     xp_c[:, :, k * BH:k * BH + CHUNK],
                                     start=(k == 0), stop=(k == KD - 1),
                                     perf_mode=mybir.MatmulPerfMode.DoubleRow)

                s2 = xs.tile([IC, CHUNK], F32, tag="s2")
                sl = At[r][:]
                nc.scalar.activation(s2[:], pt2[:], ACT.Copy, scale=float(1.0 / SC))
                nc.gpsimd.tensor_tensor(sl, sl, s2[:], OP.add)
                nc.gpsimd.tensor_scalar(sl, sl, av2, bv2, OP.mult, OP.add)

            # ---- LIF scan in v = U/beta space: two 8-wide chains, 4 DVE
            # instrs per step, every RAW producer 2 instructions back ----
            HB = BH // 2
            for t in range(T):
                rt, lt = t // TPC, (t % TPC) * BH
                d0 = St[rt][:, lt:lt + HB]
                d1 = St[rt][:, lt + HB:lt + BH]
                a0 = At[rt][:, lt:lt + HB]
                a1 = At[rt][:, lt + HB:lt + BH]
                v0_, v1_ = Vc[:, :HB], Vc[:, HB:]
                nc.vector.scalar_tensor_tensor(d0, v0_, thr, a0,
                                               OP.is_ge, OP.subtract)
                nc.vector.scalar_tensor_tensor(d1, v1_, thr, a1,
                                               OP.is_ge, OP.subtract)
                nc.vector.scalar_tensor_tensor(v0_, v0_, beta, d0,
                                               OP.mult, OP.subtract)
                nc.vector.scalar_tensor_tensor(v1_, v1_, beta, d1,
                                               OP.mult, OP.subtract)

            # ---- spike recovery + output: S_{t-1} = (d_t + A2_t >= 0.5) ----
            # Runs on Pool, trailing the scan by one chunk; exact {0,1} out.
            for r in range(NCH - 1):
                O = ostp.tile([IC, CHUNK], F32, tag="ost")
                nc.gpsimd.tensor_tensor(O[:, 0:CHUNK - BH], St[r][:, BH:],
                                        At[r][:, BH:], OP.add)
                nc.gpsimd.tensor_tensor(O[:, CHUNK - BH:], St[r + 1][:, 0:BH],
                                        At[r + 1][:, 0:BH], OP.add)
                nc.gpsimd.tensor_scalar(O[:], O[:], 0.5, None, OP.is_ge)
                nc.sync.dma_start(sout_d[:, r * CHUNK:(r + 1) * CHUNK], O[:])
            # last chunk in four 8-step pieces so the post-scan tail is tiny
            r = NCH - 1
            O = ostp.tile([IC, CHUNK], F32, tag="ost31")
            Q = CHUNK // 4
            for j in range(4):
                lo, hi = j * Q, (j + 1) * Q
                if j < 3:
                    nc.gpsimd.tensor_tensor(O[:, lo:hi], St[r][:, lo + BH:hi + BH],
                                            At[r][:, lo + BH:hi + BH], OP.add)
                    nc.gpsimd.tensor_scalar(O[:, lo:hi], O[:, lo:hi],
                                            0.5, None, OP.is_ge)
                else:
                    nc.gpsimd.tensor_tensor(O[:, lo:hi - BH], St[r][:, lo + BH:],
                                            At[r][:, lo + BH:], OP.add)
                    nc.gpsimd.tensor_scalar(O[:, lo:hi - BH], O[:, lo:hi - BH],
                                            0.5, None, OP.is_ge)
                    # closing spikes s_{T-1} = (v_{T-1} >= thr), exact
                    nc.vector.tensor_scalar(O[:, hi - BH:hi - HB], Vc[:, :HB],
                                            thr, None, OP.is_ge)
                    nc.vector.tensor_scalar(O[:, hi - HB:hi], Vc[:, HB:],
                                            thr, None, OP.is_ge)
                nc.sync.dma_start(sout_d[:, r * CHUNK + lo:r * CHUNK + hi],
                                  O[:, lo:hi])

    nc.finalize()
    return nc


def _prep_inputs(x, delay_w, delay_P, beta, bn_gamma, bn_beta, U0):
    import ml_dtypes
    f8 = ml_dtypes.float8_e4m3
    c = (delay_P.astype(np.float32) + KD // 2)
    k = np.arange(KD, dtype=np.float32)
    g = np.exp(-0.5 * ((k[None, None, :] - c[:, :, None]) / SIG) ** 2).astype(np.float32)
    g = g / (g.sum(-1, keepdims=True) + np.float32(1e-7))
    kern = (delay_w.astype(np.float32)[:, :, None] * g).astype(np.float32)  # (I,J,KD)

    kh = _to_fp32r(kern)
    kl = (kern - kh).astype(np.float32)
    xh = _to_fp32r(x)
    xl = (x - xh).astype(np.float32)

    kh_jki = np.ascontiguousarray(kh.transpose(1, 2, 0))       # (J,KD,I) f32
    kl8s = np.ascontiguousarray((kl * SC).transpose(1, 2, 0)).astype(f8)
    kh8 = kh_jki.astype(f8)

    xt_h = np.ascontiguousarray(xh.transpose(2, 0, 1))         # (J,T,B) f32
    xh8 = xt_h.astype(f8)
    xl8s = np.ascontiguousarray((xl * SC).transpose(2, 0, 1)).astype(f8)

    rb = (1.0 / beta).astype(np.float32)
    g2_full = ((1.0 - beta) * bn_gamma * rb).astype(np.float32)
    bb2_full = ((1.0 - beta) * bn_beta * rb).astype(np.float32)

    in_maps = []
    for core in range(N_CORES):
        gi, hi = core // 2, core % 2
        isl = slice(gi * IC, (gi + 1) * IC)
        bsl = slice(hi * BH, (hi + 1) * BH)
        pch = np.stack([beta[isl], rb[isl], g2_full[isl], bb2_full[isl]], axis=1)
        w8 = np.empty((J, KD, 2, IC), f8)
        w8[:, :, 0, :] = kl8s[:, :, isl]
        w8[:, :, 1, :] = kh8[:, :, isl]
        xp = np.empty((J, 2, ROWS), f8)
        xp[:, 0, :] = xh8[:, :, bsl].reshape(J, ROWS)
        xp[:, 1, :] = xl8s[:, :, bsl].reshape(J, ROWS)
        in_maps.append({
            "xh": np.ascontiguousarray(xt_h[:, :, bsl]).reshape(J, ROWS),
            "xp": xp,
            "whr": np.ascontiguousarray(kh_jki[:, :, isl]).reshape(J, KD * IC),
            "w8": np.ascontiguousarray(w8),
            "v0": np.ascontiguousarray((U0[bsl, isl] * rb[None, isl]).T.astype(np.float32)),
            "pch": np.ascontiguousarray(pch.astype(np.float32)),
        })
    return in_maps


def run_spmd(in_maps, **kwargs):
    from concourse.bass_utils import run_bass_kernel_spmd
    if "nc" not in _CACHE:
        _CACHE["nc"] = _build_nc()
    return run_bass_kernel_spmd(_CACHE["nc"], in_maps,
                                core_ids=list(range(N_CORES)), **kwargs)


def kernel(x, delay_w, delay_P, beta, bn_gamma, bn_beta, U0):
    in_maps = _prep_inputs(np.asarray(x, np.float32), np.asarray(delay_w, np.float32),
                           np.asarray(delay_P, np.float32), np.asarray(beta, np.float32),
                           np.asarray(bn_gamma, np.float32), np.asarray(bn_beta, np.float32),
                           np.asarray(U0, np.float32))
    res = run_spmd(in_maps)
    out = np.empty((T, B, I), np.float32)
    for core in range(N_CORES):
        gi, hi = core // 2, core % 2
        s = res.results[core]["sout"].reshape(IC, T, BH)
        out[:, hi * BH:(hi + 1) * BH, gi * IC:(gi + 1) * IC] = s.transpose(1, 2, 0)
    return out


# revision 15
# speedup vs baseline: 1.0756x; 1.0016x over previous
"""DelayLMLIFLayer Trainium2 kernel.

Pipeline per core (8 cores, 4-way I-shard x 2-way B-shard):
  1. DCLS delayed conv main term: 16 time-shifted fp32r matmuls per chunk,
     PSUM-accumulated; doubles as the BatchNorm stats source (accum_out).
  2. BN stats: pairwise AllGather over b-half pairs + local add.
  3. Cross-term correction in ONE fp8e4 DoubleRow pass: each tap matmul
     computes wl@xh + wh@xl simultaneously (pair-packed operands, x2^11
     scaling to keep the low parts in fp8 range), at 0.5 cycles/row.
  4. Scan runs in v = U/beta space, 2 DVE instrs per step per b-half chain:
       d_t = (v >= 1/beta) - A2_t        (= S_{t-1} - A2_t)
       v'  = beta*v - d_t                (= beta*v - S_{t-1} + A2_t)
     Two 8-wide chains interleave so every producer is 2 instructions back,
     hiding the SBUF write-ack + semaphore latency of each RAW edge.
     Spikes are recovered off the critical path on Pool:
       S_{t-1} = ((d_t + A2_t) >= 0.5), exact {0,1}.
  5. A burst of dummy matmuls during the initial weight-DMA shadow absorbs
     the PE p-state ramp so pass-1 runs at full clock throughout.
Host does layout transposes, fp32r/fp8 splits, and beta-space folds.
"""
import sys
sys.path.insert(0, '/opt/trn_rl_repo')

import numpy as np

T, B, J, I, KD = 1024, 32, 128, 512, 16
SIG = 0.5
EPS = 1e-5
N_CORES = 8
BH = B // 2          # batch elems per core (b-half)
IC = 128             # channels per core (I-chunk)
ROWS = T * BH        # free-dim rows per core
PAD = (KD - 1) * BH  # left zero pad columns (240)
CHUNK = 512          # psum tile free size
NCH = ROWS // CHUNK  # 32 row chunks
TPC = CHUNK // BH    # 32 timesteps per chunk
SC = 2.0 ** 11       # fp8 low-part scale

_CACHE = {}


def _to_fp32r(x):
    u = np.ascontiguousarray(x, np.float32).view(np.uint32).astype(np.uint64)
    rnd = ((u >> 12) & 1) + 0x7FF
    u = ((u + rnd) >> 12) << 12
    return (u & 0xFFFFFFFF).astype(np.uint32).view(np.float32)


def _build_nc():
    import concourse.bacc as bacc
    import concourse.mybir as mybir
    import concourse.tile as tile

    F32 = mybir.dt.float32
    F32R = mybir.dt.float32r
    F8 = mybir.dt.float8e4
    BF16 = mybir.dt.bfloat16
    OP = mybir.AluOpType
    ACT = mybir.ActivationFunctionType

    nc = bacc.Bacc("TRN2", target_bir_lowering=False, debug=False,
                   num_devices=N_CORES)

    xh_d = nc.dram_tensor("xh", [J, ROWS], F32, kind="ExternalInput")
    xp_d = nc.dram_tensor("xp", [J, 2, ROWS], F8, kind="ExternalInput")
    whr_d = nc.dram_tensor("whr", [J, KD * IC], F32, kind="ExternalInput")
    w8_d = nc.dram_tensor("w8", [J, KD, 2, IC], F8, kind="ExternalInput")
    v0_d = nc.dram_tensor("v0", [IC, BH], F32, kind="ExternalInput")
    pch_d = nc.dram_tensor("pch", [IC, 4], F32, kind="ExternalInput")
    sout_d = nc.dram_tensor("sout", [IC, ROWS], F32, kind="ExternalOutput")

    with tile.TileContext(nc) as tc:
        with (
            tc.tile_pool(name="big", bufs=1) as big,
            tc.tile_pool(name="xs", bufs=3) as xs,
            tc.tile_pool(name="ost", bufs=2) as ostp,
            tc.tile_pool(name="small", bufs=1) as small,
            tc.tile_pool(name="ps", bufs=4, space="PSUM") as ps,
            tc.tile_pool(name="dram", bufs=1, space="DRAM") as dram,
        ):
            At = [big.tile([IC, CHUNK], F32, tag=f"A{r}", name=f"A{r}")
                  for r in range(NCH)]
            St = [big.tile([IC, CHUNK], F32, tag=f"S{r}", name=f"S{r}")
                  for r in range(NCH)]
            scr = big.tile([IC, CHUNK], F32, tag="scr")
            whr = small.tile([J, KD * IC], F32R, tag="whr")
            w8 = small.tile([J, KD, 2, IC], F8, tag="w8")
            pch = small.tile([IC, 4], F32, tag="pch")
            Vc = small.tile([IC, BH], F32, tag="Vc")
            ssum = small.tile([IC, NCH], F32, tag="ssum")
            ssq = small.tile([IC, NCH], F32, tag="ssq")
            st2 = small.tile([IC, 2], F32, tag="st2")
            gs4 = small.tile([IC, 4], F32, tag="gs4")
            gs = small.tile([IC, 2], F32, tag="gs")
            prm = small.tile([IC, 8], F32, tag="prm")

            cc_in = dram.tile([IC, 2], F32)
            cc_out = dram.tile([2, IC, 2], F32)

            beta = pch[:, 0:1]
            thr = pch[:, 1:2]
            g2 = pch[:, 2:3]
            bb2 = pch[:, 3:4]

            # chunk-0 x rides first on the DMA queue, then pass-1 weights in
            # quarters/half so taps 0-3 gate as soon as possible
            xh_c0 = xs.tile([J, PAD + CHUNK], F32R, tag="xh_c")
            nc.vector.memset(xh_c0[:, :PAD].bitcast(F32), 0.0)
            nc.sync.dma_start(xh_c0[:, PAD:], xh_d[:, 0:CHUNK].bitcast(F32R))
            nc.sync.dma_start(whr[:, :4 * IC], whr_d[:, :4 * IC].bitcast(F32R))
            nc.sync.dma_start(whr[:, 4 * IC:8 * IC],
                              whr_d[:, 4 * IC:8 * IC].bitcast(F32R))
            nc.sync.dma_start(whr[:, 8 * IC:], whr_d[:, 8 * IC:].bitcast(F32R))

            # PE p-state warmup: dummy matmuls on zeroed tiles burn the
            # low/mid-clock ramp inside the initial DMA shadow. The first 8
            # run 512 cols (>3us of engine time); the rest are 2-col fillers
            # that keep the PE queue full so every real matmul is costed at
            # the ramped clock.
            wz = small.tile([J, 2], BF16, tag="wz")
            xz = small.tile([J, CHUNK], BF16, tag="xz")
            nc.vector.memset(wz[:], 0.0)
            nc.vector.memset(xz[:], 0.0)
            ptw = ps.tile([IC, CHUNK], F32, tag="pt")
            for i in range(8):
                nc.tensor.matmul(ptw[0:2, :], wz[:], xz[:], start=True, stop=True)
            for i in range(30):
                nc.tensor.matmul(ptw[0:2, 0:2], wz[:], xz[:, 0:2],
                                 start=True, stop=True)

            # ---- conv pass 1: fp32r main term; doubles as the BN stats source ----
            for r in range(NCH):
                c0 = r * CHUNK - PAD
                if r == 0:
                    xh_c = xh_c0
                else:
                    xh_c = xs.tile([J, PAD + CHUNK], F32R, tag="xh_c")
                    nc.sync.dma_start(xh_c[:], xh_d[:, c0:c0 + PAD + CHUNK].bitcast(F32R))
                if r == 1:
                    # scan/pass-2 constants ride behind the first two x chunks
                    nc.sync.dma_start(Vc[:], v0_d[:])
                    nc.sync.dma_start(pch[:], pch_d[:])
                    nc.sync.dma_start(w8[:], w8_d[:])

                pt = ps.tile([IC, CHUNK], F32, tag="pt")
                for k in range(KD):
                    nc.tensor.matmul(pt[:], whr[:, k * IC:(k + 1) * IC],
                                     xh_c[:, k * BH:k * BH + CHUNK],
                                     start=(k == 0), stop=(k == KD - 1))

                nc.scalar.activation(At[r][:], pt[:], ACT.Copy,
                                     accum_out=ssum[:, r:r + 1])
                nc.scalar.activation(scr[:], pt[:], ACT.Square,
                                     accum_out=ssq[:, r:r + 1])

            # ---- BN stats allreduce over the b-half pair ----
            nc.vector.tensor_reduce(st2[:, 0:1], ssum[:], mybir.AxisListType.X, OP.add)
            nc.vector.tensor_reduce(st2[:, 1:2], ssq[:], mybir.AxisListType.X, OP.add)
            nc.sync.dma_start(cc_in[:], st2[:])
            # AllGather + local add == AllReduce (add is commutative) at
            # roughly half the fixed latency.
            nc.gpsimd.collective_compute(
                "AllGather", OP.bypass,
                replica_groups=[[0, 1], [2, 3], [4, 5], [6, 7]],
                ins=[cc_in.opt()], outs=[cc_out.opt()],
            )
            # parallel queues: SP and ACT each fetch one gathered half
            nc.sync.dma_start(gs4[:, 0:2], cc_out[0, :, :])
            nc.scalar.dma_start(gs4[:, 2:4], cc_out[1, :, :])
            nc.vector.tensor_tensor(gs[:], gs4[:, 0:2], gs4[:, 2:4], OP.add)

            # ---- fold BN + (1-beta) + 1/beta into per-channel av2, bv2 ----
            inv_n = 1.0 / (T * B)
            mean = prm[:, 0:1]; ey2 = prm[:, 1:2]; var = prm[:, 2:3]
            rs = prm[:, 3:4]; av2 = prm[:, 4:5]; bv2 = prm[:, 5:6]
            tmp = prm[:, 6:7]
            nc.vector.tensor_scalar(mean, gs[:, 0:1], inv_n, None, OP.mult)
            nc.vector.tensor_scalar(ey2, gs[:, 1:2], inv_n, None, OP.mult)
            nc.vector.tensor_tensor(tmp, mean, mean, OP.mult)
            nc.vector.tensor_tensor(var, ey2, tmp, OP.subtract)
            nc.vector.tensor_scalar(var, var, EPS, None, OP.add)
            nc.scalar.sqrt(tmp, var)
            nc.vector.reciprocal(rs, tmp)
            nc.vector.tensor_tensor(av2, g2, rs, OP.mult)       # av2 = (1-b)g/(b*sigma)
            nc.vector.tensor_tensor(tmp, av2, mean, OP.mult)
            nc.vector.tensor_tensor(bv2, bb2, tmp, OP.subtract)  # bv2 = bb2 - av2*mean

            # ---- conv pass 2: fp8 DoubleRow cross terms + combine + affine.
            # Runs on PE/ACT/Pool concurrently with the DVE scan below.
            for r in range(NCH):
                c0 = r * CHUNK - PAD
                xp_c = xs.tile([J, 2, PAD + CHUNK], F8, tag="xp_c")
                if r == 0:
                    nc.vector.memset(xp_c[:, :, :PAD], 0.0)
                    nc.sync.dma_start(xp_c[:, :, PAD:], xp_d[:, :, 0:CHUNK])
                else:
                    nc.sync.dma_start(xp_c[:], xp_d[:, :, c0:c0 + PAD + CHUNK])

                pt2 = ps.tile([IC, CHUNK], F32, tag="pt2")
                for k in range(KD):
                    nc.tensor.matmul(pt2[:], w8[:, k, :, :],
                                     xp_c[:, :, k * BH:k * BH + CHUNK],
                                     start=(k == 0), stop=(k == KD - 1),
                                     perf_mode=mybir.MatmulPerfMode.DoubleRow)

                s2 = xs.tile([IC, CHUNK], F32, tag="s2")
                sl = At[r][:]
                nc.scalar.activation(s2[:], pt2[:], ACT.Copy, scale=float(1.0 / SC))
                if r == 0:
                    # chunk 0 combines on DVE right behind the param chain so
                    # the scan isn't gated on a cross-engine Pool round trip
                    nc.vector.tensor_tensor(sl, sl, s2[:], OP.add)
                    nc.vector.tensor_scalar(sl, sl, av2, bv2, OP.mult, OP.add)
                else:
                    nc.gpsimd.tensor_tensor(sl, sl, s2[:], OP.add)
                    nc.gpsimd.tensor_scalar(sl, sl, av2, bv2, OP.mult, OP.add)

            # ---- LIF scan in v = U/beta space: two 8-wide chains, 4 DVE
            # instrs per step, every RAW producer 2 instructions back ----
            HB = BH // 2
            for t in range(T):
                rt, lt = t // TPC, (t % TPC) * BH
                d0 = St[rt][:, lt:lt + HB]
                d1 = St[rt][:, lt + HB:lt + BH]
                a0 = At[rt][:, lt:lt + HB]
                a1 = At[rt][:, lt + HB:lt + BH]
                v0_, v1_ = Vc[:, :HB], Vc[:, HB:]
                nc.vector.scalar_tensor_tensor(d0, v0_, thr, a0,
                                               OP.is_ge, OP.subtract)
                nc.vector.scalar_tensor_tensor(d1, v1_, thr, a1,
                                               OP.is_ge, OP.subtract)
                nc.vector.scalar_tensor_tensor(v0_, v0_, beta, d0,
                                               OP.mult, OP.subtract)
                nc.vector.scalar_tensor_tensor(v1_, v1_, beta, d1,
                                               OP.mult, OP.subtract)

            # ---- spike recovery + output: S_{t-1} = (d_t + A2_t >= 0.5) ----
            # Runs on Pool, trailing the scan by one chunk; exact {0,1} out.
            for r in range(NCH - 1):
                O = ostp.tile([IC, CHUNK], F32, tag="ost")
                nc.gpsimd.tensor_tensor(O[:, 0:CHUNK - BH], St[r][:, BH:],
                                        At[r][:, BH:], OP.add)
                nc.gpsimd.tensor_tensor(O[:, CHUNK - BH:], St[r + 1][:, 0:BH],
                                        At[r + 1][:, 0:BH], OP.add)
                nc.gpsimd.tensor_scalar(O[:], O[:], 0.5, None, OP.is_ge)
                nc.sync.dma_start(sout_d[:, r * CHUNK:(r + 1) * CHUNK], O[:])
            # last chunk in four 8-step pieces so the post-scan tail is tiny
            r = NCH - 1
            O = ostp.tile([IC, CHUNK], F32, tag="ost31")
            Q = CHUNK // 4
            for j in range(4):
                lo, hi = j * Q, (j + 1) * Q
                if j < 3:
                    nc.gpsimd.tensor_tensor(O[:, lo:hi], St[r][:, lo + BH:hi + BH],
                                            At[r][:, lo + BH:hi + BH], OP.add)
                    nc.gpsimd.tensor_scalar(O[:, lo:hi], O[:, lo:hi],
                                            0.5, None, OP.is_ge)
                else:
                    nc.gpsimd.tensor_tensor(O[:, lo:hi - BH], St[r][:, lo + BH:],
                                            At[r][:, lo + BH:], OP.add)
                    nc.gpsimd.tensor_scalar(O[:, lo:hi - BH], O[:, lo:hi - BH],
                                            0.5, None, OP.is_ge)
                    # closing spikes s_{T-1} = (v_{T-1} >= thr), exact
                    nc.vector.tensor_scalar(O[:, hi - BH:hi - HB], Vc[:, :HB],
                                            thr, None, OP.is_ge)
                    nc.vector.tensor_scalar(O[:, hi - HB:hi], Vc[:, HB:],
                                            thr, None, OP.is_ge)
                nc.sync.dma_start(sout_d[:, r * CHUNK + lo:r * CHUNK + hi],
                                  O[:, lo:hi])

    nc.finalize()
    return nc


def _prep_inputs(x, delay_w, delay_P, beta, bn_gamma, bn_beta, U0):
    import ml_dtypes
    f8 = ml_dtypes.float8_e4m3
    c = (delay_P.astype(np.float32) + KD // 2)
    k = np.arange(KD, dtype=np.float32)
    g = np.exp(-0.5 * ((k[None, None, :] - c[:, :, None]) / SIG) ** 2).astype(np.float32)
    g = g / (g.sum(-1, keepdims=True) + np.float32(1e-7))
    kern = (delay_w.astype(np.float32)[:, :, None] * g).astype(np.float32)  # (I,J,KD)

    kh = _to_fp32r(kern)
    kl = (kern - kh).astype(np.float32)
    xh = _to_fp32r(x)
    xl = (x - xh).astype(np.float32)

    kh_jki = np.ascontiguousarray(kh.transpose(1, 2, 0))       # (J,KD,I) f32
    kl8s = np.ascontiguousarray((kl * SC).transpose(1, 2, 0)).astype(f8)
    kh8 = kh_jki.astype(f8)

    xt_h = np.ascontiguousarray(xh.transpose(2, 0, 1))         # (J,T,B) f32
    xh8 = xt_h.astype(f8)
    xl8s = np.ascontiguousarray((xl * SC).transpose(2, 0, 1)).astype(f8)

    rb = (1.0 / beta).astype(np.float32)
    g2_full = ((1.0 - beta) * bn_gamma * rb).astype(np.float32)
    bb2_full = ((1.0 - beta) * bn_beta * rb).astype(np.float32)

    in_maps = []
    for core in range(N_CORES):
        gi, hi = core // 2, core % 2
        isl = slice(gi * IC, (gi + 1) * IC)
        bsl = slice(hi * BH, (hi + 1) * BH)
        pch = np.stack([beta[isl], rb[isl], g2_full[isl], bb2_full[isl]], axis=1)
        w8 = np.empty((J, KD, 2, IC), f8)
        w8[:, :, 0, :] = kl8s[:, :, isl]
        w8[:, :, 1, :] = kh8[:, :, isl]
        xp = np.empty((J, 2, ROWS), f8)
        xp[:, 0, :] = xh8[:, :, bsl].reshape(J, ROWS)
        xp[:, 1, :] = xl8s[:, :, bsl].reshape(J, ROWS)
        in_maps.append({
            "xh": np.ascontiguousarray(xt_h[:, :, bsl]).reshape(J, ROWS),
            "xp": xp,
            "whr": np.ascontiguousarray(kh_jki[:, :, isl]).reshape(J, KD * IC),
            "w8": np.ascontiguousarray(w8),
            "v0": np.ascontiguousarray((U0[bsl, isl] * rb[None, isl]).T.astype(np.float32)),
            "pch": np.ascontiguousarray(pch.astype(np.float32)),
        })
    return in_maps


def run_spmd(in_maps, **kwargs):
    from concourse.bass_utils import run_bass_kernel_spmd
    if "nc" not in _CACHE:
        _CACHE["nc"] = _build_nc()
    return run_bass_kernel_spmd(_CACHE["nc"], in_maps,
                                core_ids=list(range(N_CORES)), **kwargs)


def kernel(x, delay_w, delay_P, beta, bn_gamma, bn_beta, U0):
    in_maps = _prep_inputs(np.asarray(x, np.float32), np.asarray(delay_w, np.float32),
                           np.asarray(delay_P, np.float32), np.asarray(beta, np.float32),
                           np.asarray(bn_gamma, np.float32), np.asarray(bn_beta, np.float32),
                           np.asarray(U0, np.float32))
    res = run_spmd(in_maps)
    out = np.empty((T, B, I), np.float32)
    for core in range(N_CORES):
        gi, hi = core // 2, core % 2
        s = res.results[core]["sout"].reshape(IC, T, BH)
        out[:, hi * BH:(hi + 1) * BH, gi * IC:(gi + 1) * IC] = s.transpose(1, 2, 0)
    return out


# revision 19
# speedup vs baseline: 1.0777x; 1.0020x over previous
"""DelayLMLIFLayer Trainium2 kernel.

Pipeline per core (8 cores, 4-way I-shard x 2-way B-shard):
  1. DCLS delayed conv main term: 16 time-shifted fp32r matmuls per chunk,
     PSUM-accumulated; doubles as the BatchNorm stats source (accum_out).
  2. BN stats: pairwise AllGather over b-half pairs + local add.
  3. Cross-term correction in ONE fp8e4 DoubleRow pass: each tap matmul
     computes wl@xh + wh@xl simultaneously (pair-packed operands, x2^11
     scaling to keep the low parts in fp8 range), at 0.5 cycles/row.
  4. Scan runs in v = U/beta space, 2 DVE instrs per step per b-half chain:
       d_t = (v >= 1/beta) - A2_t        (= S_{t-1} - A2_t)
       v'  = beta*v - d_t                (= beta*v - S_{t-1} + A2_t)
     Two 8-wide chains interleave so every producer is 2 instructions back,
     hiding the SBUF write-ack + semaphore latency of each RAW edge.
     Spikes are recovered off the critical path on Pool:
       S_{t-1} = ((d_t + A2_t) >= 0.5), exact {0,1}.
  5. A burst of dummy matmuls during the initial weight-DMA shadow absorbs
     the PE p-state ramp so pass-1 runs at full clock throughout.
Host does layout transposes, fp32r/fp8 splits, and beta-space folds.
"""
import sys
sys.path.insert(0, '/opt/trn_rl_repo')

import numpy as np

T, B, J, I, KD = 1024, 32, 128, 512, 16
SIG = 0.5
EPS = 1e-5
N_CORES = 8
BH = B // 2          # batch elems per core (b-half)
IC = 128             # channels per core (I-chunk)
ROWS = T * BH        # free-dim rows per core
PAD = (KD - 1) * BH  # left zero pad columns (240)
CHUNK = 512          # psum tile free size
NCH = ROWS // CHUNK  # 32 row chunks
TPC = CHUNK // BH    # 32 timesteps per chunk
SC = 2.0 ** 11       # fp8 low-part scale

_CACHE = {}


def _to_fp32r(x):
    u = np.ascontiguousarray(x, np.float32).view(np.uint32).astype(np.uint64)
    rnd = ((u >> 12) & 1) + 0x7FF
    u = ((u + rnd) >> 12) << 12
    return (u & 0xFFFFFFFF).astype(np.uint32).view(np.float32)


def _build_nc():
    import concourse.bacc as bacc
    import concourse.mybir as mybir
    import concourse.tile as tile

    F32 = mybir.dt.float32
    F32R = mybir.dt.float32r
    F8 = mybir.dt.float8e4
    BF16 = mybir.dt.bfloat16
    OP = mybir.AluOpType
    ACT = mybir.ActivationFunctionType

    nc = bacc.Bacc("TRN2", target_bir_lowering=False, debug=False,
                   num_devices=N_CORES)

    xh_d = nc.dram_tensor("xh", [J, ROWS], F32, kind="ExternalInput")
    xp_d = nc.dram_tensor("xp", [J, 2, ROWS], F8, kind="ExternalInput")
    whr_d = nc.dram_tensor("whr", [J, KD * IC], F32, kind="ExternalInput")
    w8_d = nc.dram_tensor("w8", [J, KD, 2, IC], F8, kind="ExternalInput")
    v0_d = nc.dram_tensor("v0", [IC, BH], F32, kind="ExternalInput")
    pch_d = nc.dram_tensor("pch", [IC, 4], F32, kind="ExternalInput")
    sout_d = nc.dram_tensor("sout", [IC, ROWS], F32, kind="ExternalOutput")

    with tile.TileContext(nc) as tc:
        with (
            tc.tile_pool(name="big", bufs=1) as big,
            tc.tile_pool(name="xs", bufs=3) as xs,
            tc.tile_pool(name="ost", bufs=2) as ostp,
            tc.tile_pool(name="small", bufs=1) as small,
            tc.tile_pool(name="ps", bufs=4, space="PSUM") as ps,
            tc.tile_pool(name="dram", bufs=1, space="DRAM") as dram,
        ):
            At = [big.tile([IC, CHUNK], F32, tag=f"A{r}", name=f"A{r}")
                  for r in range(NCH)]
            St = [big.tile([IC, CHUNK], F32, tag=f"S{r}", name=f"S{r}")
                  for r in range(NCH)]
            scr = big.tile([IC, CHUNK], F32, tag="scr")
            whr = small.tile([J, KD * IC], F32R, tag="whr")
            w8 = small.tile([J, KD, 2, IC], F8, tag="w8")
            pch = small.tile([IC, 4], F32, tag="pch")
            Vc = small.tile([IC, BH], F32, tag="Vc")
            ssum = small.tile([IC, NCH], F32, tag="ssum")
            ssq = small.tile([IC, NCH], F32, tag="ssq")
            st2 = small.tile([IC, 2], F32, tag="st2")
            gs4 = small.tile([IC, 4], F32, tag="gs4")
            gs = small.tile([IC, 2], F32, tag="gs")
            prm = small.tile([IC, 8], F32, tag="prm")

            cc_in = dram.tile([IC, 2], F32)
            cc_out = dram.tile([2, IC, 2], F32)

            beta = pch[:, 0:1]
            thr = pch[:, 1:2]
            g2 = pch[:, 2:3]
            bb2 = pch[:, 3:4]

            # chunk-0 x rides first on the DMA queue, then pass-1 weights in
            # quarters/half so taps 0-3 gate as soon as possible
            xh_c0 = xs.tile([J, PAD + CHUNK], F32R, tag="xh_c")
            nc.vector.memset(xh_c0[:, :PAD].bitcast(F32), 0.0)
            nc.sync.dma_start(xh_c0[:, PAD:], xh_d[:, 0:CHUNK].bitcast(F32R))
            nc.sync.dma_start(whr[:, :4 * IC], whr_d[:, :4 * IC].bitcast(F32R))
            nc.sync.dma_start(whr[:, 4 * IC:8 * IC],
                              whr_d[:, 4 * IC:8 * IC].bitcast(F32R))
            nc.sync.dma_start(whr[:, 8 * IC:], whr_d[:, 8 * IC:].bitcast(F32R))

            # PE p-state warmup: dummy matmuls on zeroed tiles burn the
            # low/mid-clock ramp inside the initial DMA shadow. The first 8
            # run 512 cols (>3us of engine time); the rest are 2-col fillers
            # that keep the PE queue full so every real matmul is costed at
            # the ramped clock.
            wz = small.tile([J, 2], BF16, tag="wz")
            xz = small.tile([J, CHUNK], BF16, tag="xz")
            sq0 = small.tile([J, 1], F32, tag="sq0")
            nc.vector.memset(wz[:], 0.0)
            nc.vector.memset(xz[:], 0.0)
            nc.vector.memset(sq0[:], 0.0)
            # dummy sqrt pulls the ACT sqrt-table load into the DMA shadow;
            # all other ACT use is Copy (present in every table set), so the
            # table is still resident when the BN fold needs the real sqrt
            nc.scalar.sqrt(sq0[:], sq0[:])
            ptw = ps.tile([IC, CHUNK], F32, tag="pt")
            for i in range(8):
                nc.tensor.matmul(ptw[0:2, :], wz[:], xz[:], start=True, stop=True)
            for i in range(30):
                nc.tensor.matmul(ptw[0:2, 0:2], wz[:], xz[:, 0:2],
                                 start=True, stop=True)

            # ---- conv pass 1: fp32r main term; doubles as the BN stats source ----
            for r in range(NCH):
                c0 = r * CHUNK - PAD
                if r == 0:
                    xh_c = xh_c0
                else:
                    xh_c = xs.tile([J, PAD + CHUNK], F32R, tag="xh_c")
                    nc.sync.dma_start(xh_c[:], xh_d[:, c0:c0 + PAD + CHUNK].bitcast(F32R))
                if r == 1:
                    # scan/pass-2 constants ride behind the first two x chunks
                    nc.sync.dma_start(Vc[:], v0_d[:])
                    nc.sync.dma_start(pch[:], pch_d[:])
                    nc.sync.dma_start(w8[:], w8_d[:])

                pt = ps.tile([IC, CHUNK], F32, tag="pt")
                for k in range(KD):
                    nc.tensor.matmul(pt[:], whr[:, k * IC:(k + 1) * IC],
                                     xh_c[:, k * BH:k * BH + CHUNK],
                                     start=(k == 0), stop=(k == KD - 1))

                nc.scalar.activation(At[r][:], pt[:], ACT.Copy,
                                     accum_out=ssum[:, r:r + 1])
                # sum-of-squares on the otherwise-idle DVE keeps ACT all-Copy
                # (reads the SBUF copy; both stt sources in PSUM are illegal)
                nc.vector.scalar_tensor_tensor(scr[:], At[r][:], 1.0, At[r][:],
                                               OP.mult, OP.mult,
                                               accum_out=ssq[:, r:r + 1])

            # ---- BN stats allreduce over the b-half pair ----
            nc.vector.tensor_reduce(st2[:, 0:1], ssum[:], mybir.AxisListType.X, OP.add)
            nc.vector.tensor_reduce(st2[:, 1:2], ssq[:], mybir.AxisListType.X, OP.add)
            nc.sync.dma_start(cc_in[:], st2[:])
            # AllGather + local add == AllReduce (add is commutative) at
            # roughly half the fixed latency.
            nc.gpsimd.collective_compute(
                "AllGather", OP.bypass,
                replica_groups=[[0, 1], [2, 3], [4, 5], [6, 7]],
                ins=[cc_in.opt()], outs=[cc_out.opt()],
            )
            nc.sync.dma_start(gs4[:, 0:2], cc_out[0, :, :])
            nc.sync.dma_start(gs4[:, 2:4], cc_out[1, :, :])
            nc.vector.tensor_tensor(gs[:], gs4[:, 0:2], gs4[:, 2:4], OP.add)

            # ---- fold BN + (1-beta) + 1/beta into per-channel av2, bv2 ----
            inv_n = 1.0 / (T * B)
            mean = prm[:, 0:1]; ey2 = prm[:, 1:2]; var = prm[:, 2:3]
            rs = prm[:, 3:4]; av2 = prm[:, 4:5]; bv2 = prm[:, 5:6]
            tmp = prm[:, 6:7]
            nc.vector.tensor_scalar(mean, gs[:, 0:1], inv_n, None, OP.mult)
            nc.vector.tensor_scalar(ey2, gs[:, 1:2], inv_n, None, OP.mult)
            nc.vector.tensor_tensor(tmp, mean, mean, OP.mult)
            nc.vector.tensor_tensor(var, ey2, tmp, OP.subtract)
            nc.vector.tensor_scalar(var, var, EPS, None, OP.add)
            nc.scalar.sqrt(tmp, var)
            nc.vector.reciprocal(rs, tmp)
            nc.vector.tensor_tensor(av2, g2, rs, OP.mult)       # av2 = (1-b)g/(b*sigma)
            nc.vector.tensor_tensor(tmp, av2, mean, OP.mult)
            nc.vector.tensor_tensor(bv2, bb2, tmp, OP.subtract)  # bv2 = bb2 - av2*mean

            # ---- conv pass 2: fp8 DoubleRow cross terms + combine + affine.
            # Runs on PE/ACT/Pool concurrently with the DVE scan below.
            for r in range(NCH):
                c0 = r * CHUNK - PAD
                xp_c = xs.tile([J, 2, PAD + CHUNK], F8, tag="xp_c")
                if r == 0:
                    nc.vector.memset(xp_c[:, :, :PAD], 0.0)
                    nc.sync.dma_start(xp_c[:, :, PAD:], xp_d[:, :, 0:CHUNK])
                else:
                    nc.sync.dma_start(xp_c[:], xp_d[:, :, c0:c0 + PAD + CHUNK])

                pt2 = ps.tile([IC, CHUNK], F32, tag="pt2")
                for k in range(KD):
                    nc.tensor.matmul(pt2[:], w8[:, k, :, :],
                                     xp_c[:, :, k * BH:k * BH + CHUNK],
                                     start=(k == 0), stop=(k == KD - 1),
                                     perf_mode=mybir.MatmulPerfMode.DoubleRow)

                s2 = xs.tile([IC, CHUNK], F32, tag="s2")
                sl = At[r][:]
                nc.scalar.activation(s2[:], pt2[:], ACT.Copy, scale=float(1.0 / SC))
                if r == 0:
                    # chunk 0 combines on DVE right behind the param chain so
                    # the scan isn't gated on a cross-engine Pool round trip
                    nc.vector.tensor_tensor(sl, sl, s2[:], OP.add)
                    nc.vector.tensor_scalar(sl, sl, av2, bv2, OP.mult, OP.add)
                else:
                    nc.gpsimd.tensor_tensor(sl, sl, s2[:], OP.add)
                    nc.gpsimd.tensor_scalar(sl, sl, av2, bv2, OP.mult, OP.add)

            # ---- LIF scan in v = U/beta space: two 8-wide chains, 4 DVE
            # instrs per step, every RAW producer 2 instructions back ----
            HB = BH // 2
            for t in range(T):
                rt, lt = t // TPC, (t % TPC) * BH
                d0 = St[rt][:, lt:lt + HB]
                d1 = St[rt][:, lt + HB:lt + BH]
                a0 = At[rt][:, lt:lt + HB]
                a1 = At[rt][:, lt + HB:lt + BH]
                v0_, v1_ = Vc[:, :HB], Vc[:, HB:]
                nc.vector.scalar_tensor_tensor(d0, v0_, thr, a0,
                                               OP.is_ge, OP.subtract)
                nc.vector.scalar_tensor_tensor(d1, v1_, thr, a1,
                                               OP.is_ge, OP.subtract)
                nc.vector.scalar_tensor_tensor(v0_, v0_, beta, d0,
                                               OP.mult, OP.subtract)
                nc.vector.scalar_tensor_tensor(v1_, v1_, beta, d1,
                                               OP.mult, OP.subtract)

            # ---- spike recovery + output: S_{t-1} = (d_t + A2_t >= 0.5) ----
            # Runs on Pool, trailing the scan by one chunk; exact {0,1} out.
            for r in range(NCH - 1):
                O = ostp.tile([IC, CHUNK], F32, tag="ost")
                nc.gpsimd.tensor_tensor(O[:, 0:CHUNK - BH], St[r][:, BH:],
                                        At[r][:, BH:], OP.add)
                nc.gpsimd.tensor_tensor(O[:, CHUNK - BH:], St[r + 1][:, 0:BH],
                                        At[r + 1][:, 0:BH], OP.add)
                nc.gpsimd.tensor_scalar(O[:], O[:], 0.5, None, OP.is_ge)
                nc.sync.dma_start(sout_d[:, r * CHUNK:(r + 1) * CHUNK], O[:])
            # last chunk in four 8-step pieces so the post-scan tail is tiny
            r = NCH - 1
            O = ostp.tile([IC, CHUNK], F32, tag="ost31")
            Q = CHUNK // 4
            for j in range(4):
                lo, hi = j * Q, (j + 1) * Q
                if j < 3:
                    nc.gpsimd.tensor_tensor(O[:, lo:hi], St[r][:, lo + BH:hi + BH],
                                            At[r][:, lo + BH:hi + BH], OP.add)
                    nc.gpsimd.tensor_scalar(O[:, lo:hi], O[:, lo:hi],
                                            0.5, None, OP.is_ge)
                else:
                    nc.gpsimd.tensor_tensor(O[:, lo:hi - BH], St[r][:, lo + BH:],
                                            At[r][:, lo + BH:], OP.add)
                    nc.gpsimd.tensor_scalar(O[:, lo:hi - BH], O[:, lo:hi - BH],
                                            0.5, None, OP.is_ge)
                    # closing spikes s_{T-1} = (v_{T-1} >= thr), exact
                    nc.vector.tensor_scalar(O[:, hi - BH:hi - HB], Vc[:, :HB],
                                            thr, None, OP.is_ge)
                    nc.vector.tensor_scalar(O[:, hi - HB:hi], Vc[:, HB:],
                                            thr, None, OP.is_ge)
                nc.sync.dma_start(sout_d[:, r * CHUNK + lo:r * CHUNK + hi],
                                  O[:, lo:hi])

    nc.finalize()
    return nc


def _prep_inputs(x, delay_w, delay_P, beta, bn_gamma, bn_beta, U0):
    import ml_dtypes
    f8 = ml_dtypes.float8_e4m3
    c = (delay_P.astype(np.float32) + KD // 2)
    k = np.arange(KD, dtype=np.float32)
    g = np.exp(-0.5 * ((k[None, None, :] - c[:, :, None]) / SIG) ** 2).astype(np.float32)
    g = g / (g.sum(-1, keepdims=True) + np.float32(1e-7))
    kern = (delay_w.astype(np.float32)[:, :, None] * g).astype(np.float32)  # (I,J,KD)

    kh = _to_fp32r(kern)
    kl = (kern - kh).astype(np.float32)
    xh = _to_fp32r(x)
    xl = (x - xh).astype(np.float32)

    kh_jki = np.ascontiguousarray(kh.transpose(1, 2, 0))       # (J,KD,I) f32
    kl8s = np.ascontiguousarray((kl * SC).transpose(1, 2, 0)).astype(f8)
    kh8 = kh_jki.astype(f8)

    xt_h = np.ascontiguousarray(xh.transpose(2, 0, 1))         # (J,T,B) f32
    xh8 = xt_h.astype(f8)
    xl8s = np.ascontiguousarray((xl * SC).transpose(2, 0, 1)).astype(f8)

    rb = (1.0 / beta).astype(np.float32)
    g2_full = ((1.0 - beta) * bn_gamma * rb).astype(np.float32)
    bb2_full = ((1.0 - beta) * bn_beta * rb).astype(np.float32)

    in_maps = []
    for core in range(N_CORES):
        gi, hi = core // 2, core % 2
        isl = slice(gi * IC, (gi + 1) * IC)
        bsl = slice(hi * BH, (hi + 1) * BH)
        pch = np.stack([beta[isl], rb[isl], g2_full[isl], bb2_full[isl]], axis=1)
        w8 = np.empty((J, KD, 2, IC), f8)
        w8[:, :, 0, :] = kl8s[:, :, isl]
        w8[:, :, 1, :] = kh8[:, :, isl]
        xp = np.empty((J, 2, ROWS), f8)
        xp[:, 0, :] = xh8[:, :, bsl].reshape(J, ROWS)
        xp[:, 1, :] = xl8s[:, :, bsl].reshape(J, ROWS)
        in_maps.append({
            "xh": np.ascontiguousarray(xt_h[:, :, bsl]).reshape(J, ROWS),
            "xp": xp,
            "whr": np.ascontiguousarray(kh_jki[:, :, isl]).reshape(J, KD * IC),
            "w8": np.ascontiguousarray(w8),
            "v0": np.ascontiguousarray((U0[bsl, isl] * rb[None, isl]).T.astype(np.float32)),
            "pch": np.ascontiguousarray(pch.astype(np.float32)),
        })
    return in_maps


def run_spmd(in_maps, **kwargs):
    from concourse.bass_utils import run_bass_kernel_spmd
    if "nc" not in _CACHE:
        _CACHE["nc"] = _build_nc()
    return run_bass_kernel_spmd(_CACHE["nc"], in_maps,
                                core_ids=list(range(N_CORES)), **kwargs)


def kernel(x, delay_w, delay_P, beta, bn_gamma, bn_beta, U0):
    in_maps = _prep_inputs(np.asarray(x, np.float32), np.asarray(delay_w, np.float32),
                           np.asarray(delay_P, np.float32), np.asarray(beta, np.float32),
                           np.asarray(bn_gamma, np.float32), np.asarray(bn_beta, np.float32),
                           np.asarray(U0, np.float32))
    res = run_spmd(in_maps)
    out = np.empty((T, B, I), np.float32)
    for core in range(N_CORES):
        gi, hi = core // 2, core % 2
        s = res.results[core]["sout"].reshape(IC, T, BH)
        out[:, hi * BH:(hi + 1) * BH, gi * IC:(gi + 1) * IC] = s.transpose(1, 2, 0)
    return out


# revision 26
# speedup vs baseline: 1.0781x; 1.0003x over previous
"""DelayLMLIFLayer Trainium2 kernel.

Pipeline per core (8 cores, 4-way I-shard x 2-way B-shard):
  1. DCLS delayed conv main term: 16 time-shifted fp32r matmuls per chunk,
     PSUM-accumulated; doubles as the BatchNorm stats source (accum_out).
  2. BN stats: pairwise AllGather over b-half pairs + local add.
  3. Cross-term correction in ONE fp8e4 DoubleRow pass: each tap matmul
     computes wl@xh + wh@xl simultaneously (pair-packed operands, x2^11
     scaling to keep the low parts in fp8 range), at 0.5 cycles/row.
  4. Scan runs in v = U/beta space, 2 DVE instrs per step per b-half chain:
       d_t = (v >= 1/beta) - A2_t        (= S_{t-1} - A2_t)
       v'  = beta*v - d_t                (= beta*v - S_{t-1} + A2_t)
     Two 8-wide chains interleave so every producer is 2 instructions back,
     hiding the SBUF write-ack + semaphore latency of each RAW edge.
     Spikes are recovered off the critical path on Pool:
       S_{t-1} = ((d_t + A2_t) >= 0.5), exact {0,1}.
  5. A burst of dummy matmuls during the initial weight-DMA shadow absorbs
     the PE p-state ramp so pass-1 runs at full clock throughout.
Host does layout transposes, fp32r/fp8 splits, and beta-space folds.
"""
import sys
sys.path.insert(0, '/opt/trn_rl_repo')

import numpy as np

T, B, J, I, KD = 1024, 32, 128, 512, 16
SIG = 0.5
EPS = 1e-5
N_CORES = 8
BH = B // 2          # batch elems per core (b-half)
IC = 128             # channels per core (I-chunk)
ROWS = T * BH        # free-dim rows per core
PAD = (KD - 1) * BH  # left zero pad columns (240)
CHUNK = 512          # psum tile free size
NCH = ROWS // CHUNK  # 32 row chunks
TPC = CHUNK // BH    # 32 timesteps per chunk
SC = 2.0 ** 11       # fp8 low-part scale

_CACHE = {}


def _to_fp32r(x):
    u = np.ascontiguousarray(x, np.float32).view(np.uint32).astype(np.uint64)
    rnd = ((u >> 12) & 1) + 0x7FF
    u = ((u + rnd) >> 12) << 12
    return (u & 0xFFFFFFFF).astype(np.uint32).view(np.float32)


def _build_nc():
    import concourse.bacc as bacc
    import concourse.mybir as mybir
    import concourse.tile as tile

    F32 = mybir.dt.float32
    F32R = mybir.dt.float32r
    F8 = mybir.dt.float8e4
    BF16 = mybir.dt.bfloat16
    OP = mybir.AluOpType
    ACT = mybir.ActivationFunctionType

    nc = bacc.Bacc("TRN2", target_bir_lowering=False, debug=False,
                   num_devices=N_CORES)

    xh_d = nc.dram_tensor("xh", [J, ROWS], F32, kind="ExternalInput")
    xp_d = nc.dram_tensor("xp", [J, 2, ROWS], F8, kind="ExternalInput")
    whr_d = nc.dram_tensor("whr", [J, KD * IC], F32, kind="ExternalInput")
    w8_d = nc.dram_tensor("w8", [J, KD, 2, IC], F8, kind="ExternalInput")
    v0_d = nc.dram_tensor("v0", [IC, BH], F32, kind="ExternalInput")
    pch_d = nc.dram_tensor("pch", [IC, 4], F32, kind="ExternalInput")
    sout_d = nc.dram_tensor("sout", [IC, ROWS], F32, kind="ExternalOutput")

    with tile.TileContext(nc) as tc:
        with (
            tc.tile_pool(name="big", bufs=1) as big,
            tc.tile_pool(name="xs", bufs=3) as xs,
            tc.tile_pool(name="ost", bufs=2) as ostp,
            tc.tile_pool(name="small", bufs=1) as small,
            tc.tile_pool(name="ps", bufs=4, space="PSUM") as ps,
            tc.tile_pool(name="dram", bufs=1, space="DRAM") as dram,
        ):
            At = [big.tile([IC, CHUNK], F32, tag=f"A{r}", name=f"A{r}")
                  for r in range(NCH)]
            St = [big.tile([IC, CHUNK], F32, tag=f"S{r}", name=f"S{r}")
                  for r in range(NCH)]
            scr = big.tile([IC, CHUNK], F32, tag="scr")
            whr = small.tile([J, KD * IC], F32R, tag="whr")
            w8 = small.tile([J, KD, 2, IC], F8, tag="w8")
            pch = small.tile([IC, 4], F32, tag="pch")
            Vc = small.tile([IC, BH], F32, tag="Vc")
            ssum = small.tile([IC, NCH], F32, tag="ssum")
            ssq = small.tile([IC, NCH], F32, tag="ssq")
            st2 = small.tile([IC, 2], F32, tag="st2")
            st2p = small.tile([IC, 2], F32, tag="st2p")
            gs4 = small.tile([IC, 4], F32, tag="gs4")
            gs = small.tile([IC, 2], F32, tag="gs")
            prm = small.tile([IC, 8], F32, tag="prm")

            cc_in = dram.tile([IC, 2], F32)
            cc_out = dram.tile([2, IC, 2], F32)

            beta = pch[:, 0:1]
            thr = pch[:, 1:2]
            g2 = pch[:, 2:3]
            bb2 = pch[:, 3:4]

            # chunk-0 x rides first on the DMA queue, then pass-1 weights in
            # quarters/half so taps 0-3 gate as soon as possible
            xh_c0 = xs.tile([J, PAD + CHUNK], F32R, tag="xh_c")
            nc.vector.memset(xh_c0[:, :PAD].bitcast(F32), 0.0)
            nc.sync.dma_start(xh_c0[:, PAD:PAD + 272], xh_d[:, 0:272].bitcast(F32R))
            nc.sync.dma_start(xh_c0[:, PAD + 272:], xh_d[:, 272:CHUNK].bitcast(F32R))
            nc.sync.dma_start(whr[:, :4 * IC], whr_d[:, :4 * IC].bitcast(F32R))
            nc.sync.dma_start(whr[:, 4 * IC:8 * IC],
                              whr_d[:, 4 * IC:8 * IC].bitcast(F32R))
            nc.sync.dma_start(whr[:, 8 * IC:], whr_d[:, 8 * IC:].bitcast(F32R))

            # PE p-state warmup: dummy matmuls on zeroed tiles burn the
            # low/mid-clock ramp inside the initial DMA shadow. The first 8
            # run 512 cols (>3us of engine time); the rest are 2-col fillers
            # that keep the PE queue full so every real matmul is costed at
            # the ramped clock.
            wz = small.tile([J, 2], BF16, tag="wz")
            xz = small.tile([J, CHUNK], BF16, tag="xz")
            sq0 = small.tile([J, 1], F32, tag="sq0")
            nc.vector.memset(wz[:], 0.0)
            nc.vector.memset(xz[:], 0.0)
            nc.vector.memset(sq0[:], 0.0)
            # dummy sqrt pulls the ACT sqrt-table load into the DMA shadow;
            # all other ACT use is Copy (present in every table set), so the
            # table is still resident when the BN fold needs the real sqrt
            nc.scalar.sqrt(sq0[:], sq0[:])
            ptw = ps.tile([IC, CHUNK], F32, tag="pt")
            for i in range(8):
                nc.tensor.matmul(ptw[0:2, :], wz[:], xz[:], start=True, stop=True)
            for i in range(30):
                nc.tensor.matmul(ptw[0:2, 0:2], wz[:], xz[:, 0:2],
                                 start=True, stop=True)

            # ---- conv pass 1: fp32r main term; doubles as the BN stats source ----
            for r in range(NCH):
                c0 = r * CHUNK - PAD
                if r == 0:
                    xh_c = xh_c0
                else:
                    xh_c = xs.tile([J, PAD + CHUNK], F32R, tag="xh_c")
                    nc.sync.dma_start(xh_c[:], xh_d[:, c0:c0 + PAD + CHUNK].bitcast(F32R))
                if r == 1:
                    # scan/pass-2 constants ride behind the first two x chunks
                    nc.sync.dma_start(Vc[:], v0_d[:])
                    nc.sync.dma_start(pch[:], pch_d[:])
                    nc.sync.dma_start(w8[:], w8_d[:])
                if r == NCH - 1:
                    # pre-reduce chunks 0..30 so only the last chunk's column
                    # remains on the post-pass-1 critical path
                    nc.vector.tensor_reduce(st2p[:, 0:1], ssum[:, :NCH - 1],
                                            mybir.AxisListType.X, OP.add)
                    nc.vector.tensor_reduce(st2p[:, 1:2], ssq[:, :NCH - 1],
                                            mybir.AxisListType.X, OP.add)

                pt = ps.tile([IC, CHUNK], F32, tag="pt")
                for k in range(KD):
                    nc.tensor.matmul(pt[:], whr[:, k * IC:(k + 1) * IC],
                                     xh_c[:, k * BH:k * BH + CHUNK],
                                     start=(k == 0), stop=(k == KD - 1))

                nc.scalar.activation(At[r][:], pt[:], ACT.Copy,
                                     accum_out=ssum[:, r:r + 1])
                nc.scalar.activation(scr[:], pt[:], ACT.Square,
                                     accum_out=ssq[:, r:r + 1])

            # ---- BN stats allreduce over the b-half pair ----
            nc.vector.tensor_tensor(st2[:, 0:1], st2p[:, 0:1],
                                    ssum[:, NCH - 1:NCH], OP.add)
            nc.vector.tensor_tensor(st2[:, 1:2], st2p[:, 1:2],
                                    ssq[:, NCH - 1:NCH], OP.add)
            nc.sync.dma_start(cc_in[:], st2[:])
            # AllGather + local add == AllReduce (add is commutative) at
            # roughly half the fixed latency.
            nc.gpsimd.collective_compute(
                "AllGather", OP.bypass,
                replica_groups=[[0, 1], [2, 3], [4, 5], [6, 7]],
                ins=[cc_in.opt()], outs=[cc_out.opt()],
            )
            nc.sync.dma_start(gs4[:, 0:2], cc_out[0, :, :])
            nc.sync.dma_start(gs4[:, 2:4], cc_out[1, :, :])
            nc.vector.tensor_tensor(gs[:], gs4[:, 0:2], gs4[:, 2:4], OP.add)

            # ---- fold BN + (1-beta) + 1/beta into per-channel av2, bv2 ----
            inv_n = 1.0 / (T * B)
            mean = prm[:, 0:1]; ey2 = prm[:, 1:2]; var = prm[:, 2:3]
            rs = prm[:, 3:4]; av2 = prm[:, 4:5]; bv2 = prm[:, 5:6]
            tmp = prm[:, 6:7]
            # bv2 is carried NEGATED (av2*mean - bb2) so it folds into one
            # fused op; the affine therefore uses mult+subtract.
            nc.vector.tensor_scalar(mean, gs[:, 0:1], inv_n, None, OP.mult)
            nc.vector.tensor_scalar(ey2, gs[:, 1:2], inv_n, None, OP.mult)
            nc.vector.scalar_tensor_tensor(var, mean, mean, ey2,
                                           OP.mult, OP.subtract)  # mean^2-E[y^2]
            nc.vector.tensor_scalar(var, var, -1.0, EPS, OP.mult, OP.add)
            nc.scalar.sqrt(tmp, var)
            nc.vector.reciprocal(rs, tmp)
            nc.vector.tensor_tensor(av2, g2, rs, OP.mult)       # av2 = (1-b)g/(b*sigma)
            nc.vector.scalar_tensor_tensor(bv2, av2, mean, bb2,
                                           OP.mult, OP.subtract)  # av2*mean - bb2

            # ---- conv pass 2: fp8 DoubleRow cross terms + combine + affine.
            # Runs on PE/ACT/Pool concurrently with the DVE scan below.
            for r in range(NCH):
                c0 = r * CHUNK - PAD
                xp_c = xs.tile([J, 2, PAD + CHUNK], F8, tag="xp_c")
                if r == 0:
                    nc.vector.memset(xp_c[:, :, :PAD], 0.0)
                    nc.sync.dma_start(xp_c[:, :, PAD:], xp_d[:, :, 0:CHUNK])
                else:
                    nc.sync.dma_start(xp_c[:], xp_d[:, :, c0:c0 + PAD + CHUNK])

                pt2 = ps.tile([IC, CHUNK], F32, tag="pt2")
                for k in range(KD):
                    nc.tensor.matmul(pt2[:], w8[:, k, :, :],
                                     xp_c[:, :, k * BH:k * BH + CHUNK],
                                     start=(k == 0), stop=(k == KD - 1),
                                     perf_mode=mybir.MatmulPerfMode.DoubleRow)

                s2 = xs.tile([IC, CHUNK], F32, tag="s2")
                sl = At[r][:]
                nc.scalar.activation(s2[:], pt2[:], ACT.Copy, scale=float(1.0 / SC))
                if r == 0:
                    # chunk 0 combines on DVE right behind the param chain so
                    # the scan isn't gated on a cross-engine Pool round trip
                    nc.vector.tensor_tensor(sl, sl, s2[:], OP.add)
                    nc.vector.tensor_scalar(sl, sl, av2, bv2, OP.mult, OP.subtract)
                else:
                    nc.gpsimd.tensor_tensor(sl, sl, s2[:], OP.add)
                    nc.gpsimd.tensor_scalar(sl, sl, av2, bv2, OP.mult, OP.subtract)

            # ---- LIF scan in v = U/beta space: two 8-wide chains, 4 DVE
            # instrs per step, every RAW producer 2 instructions back ----
            HB = BH // 2
            for t in range(T):
                rt, lt = t // TPC, (t % TPC) * BH
                d0 = St[rt][:, lt:lt + HB]
                d1 = St[rt][:, lt + HB:lt + BH]
                a0 = At[rt][:, lt:lt + HB]
                a1 = At[rt][:, lt + HB:lt + BH]
                v0_, v1_ = Vc[:, :HB], Vc[:, HB:]
                nc.vector.scalar_tensor_tensor(d0, v0_, thr, a0,
                                               OP.is_ge, OP.subtract)
                nc.vector.scalar_tensor_tensor(d1, v1_, thr, a1,
                                               OP.is_ge, OP.subtract)
                nc.vector.scalar_tensor_tensor(v0_, v0_, beta, d0,
                                               OP.mult, OP.subtract)
                nc.vector.scalar_tensor_tensor(v1_, v1_, beta, d1,
                                               OP.mult, OP.subtract)

            # ---- spike recovery + output: S_{t-1} = (d_t + A2_t >= 0.5) ----
            # Runs on Pool, trailing the scan by one chunk; exact {0,1} out.
            for r in range(NCH - 1):
                O = ostp.tile([IC, CHUNK], F32, tag="ost")
                nc.gpsimd.tensor_tensor(O[:, 0:CHUNK - BH], St[r][:, BH:],
                                        At[r][:, BH:], OP.add)
                nc.gpsimd.tensor_tensor(O[:, CHUNK - BH:], St[r + 1][:, 0:BH],
                                        At[r + 1][:, 0:BH], OP.add)
                nc.gpsimd.tensor_scalar(O[:], O[:], 0.5, None, OP.is_ge)
                nc.sync.dma_start(sout_d[:, r * CHUNK:(r + 1) * CHUNK], O[:])
            # last chunk in four 8-step pieces so the post-scan tail is tiny
            r = NCH - 1
            O = ostp.tile([IC, CHUNK], F32, tag="ost31")
            Q = CHUNK // 4
            for j in range(4):
                lo, hi = j * Q, (j + 1) * Q
                if j < 3:
                    nc.gpsimd.tensor_tensor(O[:, lo:hi], St[r][:, lo + BH:hi + BH],
                                            At[r][:, lo + BH:hi + BH], OP.add)
                    nc.gpsimd.tensor_scalar(O[:, lo:hi], O[:, lo:hi],
                                            0.5, None, OP.is_ge)
                else:
                    nc.gpsimd.tensor_tensor(O[:, lo:hi - BH], St[r][:, lo + BH:],
                                            At[r][:, lo + BH:], OP.add)
                    nc.gpsimd.tensor_scalar(O[:, lo:hi - BH], O[:, lo:hi - BH],
                                            0.5, None, OP.is_ge)
                    # closing spikes s_{T-1} = (v_{T-1} >= thr), exact
                    nc.vector.tensor_scalar(O[:, hi - BH:hi - HB], Vc[:, :HB],
                                            thr, None, OP.is_ge)
                    nc.vector.tensor_scalar(O[:, hi - HB:hi], Vc[:, HB:],
                                            thr, None, OP.is_ge)
                nc.sync.dma_start(sout_d[:, r * CHUNK + lo:r * CHUNK + hi],
                                  O[:, lo:hi])

    nc.finalize()
    return nc


def _prep_inputs(x, delay_w, delay_P, beta, bn_gamma, bn_beta, U0):
    import ml_dtypes
    f8 = ml_dtypes.float8_e4m3
    c = (delay_P.astype(np.float32) + KD // 2)
    k = np.arange(KD, dtype=np.float32)
    g = np.exp(-0.5 * ((k[None, None, :] - c[:, :, None]) / SIG) ** 2).astype(np.float32)
    g = g / (g.sum(-1, keepdims=True) + np.float32(1e-7))
    kern = (delay_w.astype(np.float32)[:, :, None] * g).astype(np.float32)  # (I,J,KD)

    kh = _to_fp32r(kern)
    kl = (kern - kh).astype(np.float32)
    xh = _to_fp32r(x)
    xl = (x - xh).astype(np.float32)

    kh_jki = np.ascontiguousarray(kh.transpose(1, 2, 0))       # (J,KD,I) f32
    kl8s = np.ascontiguousarray((kl * SC).transpose(1, 2, 0)).astype(f8)
    kh8 = kh_jki.astype(f8)

    xt_h = np.ascontiguousarray(xh.transpose(2, 0, 1))         # (J,T,B) f32
    xh8 = xt_h.astype(f8)
    xl8s = np.ascontiguousarray((xl * SC).transpose(2, 0, 1)).astype(f8)

    rb = (1.0 / beta).astype(np.float32)
    g2_full = ((1.0 - beta) * bn_gamma * rb).astype(np.float32)
    bb2_full = ((1.0 - beta) * bn_beta * rb).astype(np.float32)

    in_maps = []
    for core in range(N_CORES):
        gi, hi = core // 2, core % 2
        isl = slice(gi * IC, (gi + 1) * IC)
        bsl = slice(hi * BH, (hi + 1) * BH)
        pch = np.stack([beta[isl], rb[isl], g2_full[isl], bb2_full[isl]], axis=1)
        w8 = np.empty((J, KD, 2, IC), f8)
        w8[:, :, 0, :] = kl8s[:, :, isl]
        w8[:, :, 1, :] = kh8[:, :, isl]
        xp = np.empty((J, 2, ROWS), f8)
        xp[:, 0, :] = xh8[:, :, bsl].reshape(J, ROWS)
        xp[:, 1, :] = xl8s[:, :, bsl].reshape(J, ROWS)
        in_maps.append({
            "xh": np.ascontiguousarray(xt_h[:, :, bsl]).reshape(J, ROWS),
            "xp": xp,
            "whr": np.ascontiguousarray(kh_jki[:, :, isl]).reshape(J, KD * IC),
            "w8": np.ascontiguousarray(w8),
            "v0": np.ascontiguousarray((U0[bsl, isl] * rb[None, isl]).T.astype(np.float32)),
            "pch": np.ascontiguousarray(pch.astype(np.float32)),
        })
    return in_maps


def run_spmd(in_maps, **kwargs):
    from concourse.bass_utils import run_bass_kernel_spmd
    if "nc" not in _CACHE:
        _CACHE["nc"] = _build_nc()
    return run_bass_kernel_spmd(_CACHE["nc"], in_maps,
                                core_ids=list(range(N_CORES)), **kwargs)


def kernel(x, delay_w, delay_P, beta, bn_gamma, bn_beta, U0):
    in_maps = _prep_inputs(np.asarray(x, np.float32), np.asarray(delay_w, np.float32),
                           np.asarray(delay_P, np.float32), np.asarray(beta, np.float32),
                           np.asarray(bn_gamma, np.float32), np.asarray(bn_beta, np.float32),
                           np.asarray(U0, np.float32))
    res = run_spmd(in_maps)
    out = np.empty((T, B, I), np.float32)
    for core in range(N_CORES):
        gi, hi = core // 2, core % 2
        s = res.results[core]["sout"].reshape(IC, T, BH)
        out[:, hi * BH:(hi + 1) * BH, gi * IC:(gi + 1) * IC] = s.transpose(1, 2, 0)
    return out
